# revision 1
# baseline (speedup 1.0000x reference)
"""GQA attention + RoPE + O-proj, tensor-parallel over 8 NeuronCores.

Strategy (head-parallel TP + all-to-all reshard before O-proj):
  - host: transpose x -> xT [DIM, T]; shuffle per-head wq/wk columns to
    [even hd | odd hd] so RoPE works in the transposed layout.
  - core c: projects q for heads {2c, 2c+1} and k,v for kv-head c//2 over
    all tokens (weight-stationary fp32r matmuls, xT streamed in quarter
    tiles), applies RoPE inline per token-pair (sign-folded), transposes V
    inline, then runs causal attention in S^T [k, q] layout with no-max
    softmax (scores ~N(0,1)); denominators via ones-matmul broadcast sums +
    fast Newton reciprocal.
  - Two AllToAlls (one per local head, overlapped with attention) reshard
    attention outputs head-major -> token-sharded; each core then computes
    its 512 output rows against the full wo (double-buffered halves).
"""

import os
import numpy as np

import concourse.bass as bass
import concourse.bacc as bacc
import concourse.tile as tile
from concourse import mybir
from concourse.bass_utils import run_bass_kernel_spmd

F32 = mybir.dt.float32
F32R = mybir.dt.float32r

N_CORES = 8

# Full-problem config (hardcoded per spec).
B, SB, DIM = 2, 2048, 2048         # batches, seq per batch, model dim
H, HKV, HD = 16, 4, 128            # q heads, kv heads, head dim
SCALE = 1.0 / float(np.sqrt(HD))

T = B * SB                          # 4096 flat tokens (batch-major)
TPC = T // N_CORES                  # 512 tokens per core (output shard)
HPC = H // N_CORES                  # 2 q heads per core
QW = HPC * HD                       # 256 q cols per core
NKD = DIM // 128                    # 16 contraction tiles for projections
NG = SB // 512                      # 4 q-groups of 512 per batch
KT = SB // 128                      # 16 k-tiles per batch
NTT = T // 128                      # 32 token tiles total
NHD = (H * HD) // 128               # 16 hd row-tiles of wo


def _build():
    nc = bacc.Bacc("TRN2", target_bir_lowering=False, debug=False,
                   num_devices=N_CORES)

    xT = nc.dram_tensor("xT", [DIM, T], F32R, kind="ExternalInput").ap()
    wq_c = nc.dram_tensor("wq_c", [DIM, QW], F32R, kind="ExternalInput").ap()
    wk_c = nc.dram_tensor("wk_c", [DIM, HD], F32R, kind="ExternalInput").ap()
    wv_c = nc.dram_tensor("wv_c", [DIM, HD], F32R, kind="ExternalInput").ap()
    wo_f = nc.dram_tensor("wo_f", [H * HD, DIM], F32R, kind="ExternalInput").ap()
    cosd = nc.dram_tensor("cosd", [128, SB], F32, kind="ExternalInput").ap()
    sind = nc.dram_tensor("sind", [128, SB], F32, kind="ExternalInput").ap()
    sgn = nc.dram_tensor("sgn", [128, 1], F32, kind="ExternalInput").ap()
    tri = nc.dram_tensor("tri", [128, 512], F32, kind="ExternalInput").ap()
    ones = nc.dram_tensor("ones", [128, 128], F32R, kind="ExternalInput").ap()
    ident = nc.dram_tensor("ident", [128, 128], F32R, kind="ExternalInput").ap()
    out_c = nc.dram_tensor("out_c", [TPC, DIM], F32, kind="ExternalOutput").ap()

    a2a_in = []
    a2a_out = []
    for hl in range(HPC):
        a2a_in.append(nc.dram_tensor(f"a2a_in{hl}",
                                     [N_CORES, HD, TPC], F32R).ap())
        a2a_out.append(nc.dram_tensor(f"a2a_out{hl}",
                                      [N_CORES, HD, TPC], F32R).ap())

    SEG = min(1024, SB)            # rope segment (never crosses a batch)
    NKQ = max(1, NKD // 4)         # dim-tiles per xt quarter
    NQT = NKD // NKQ               # quarters per token group

    with tile.TileContext(nc) as tc:
        with tc.tile_pool(name="const", bufs=1) as constp, \
             tc.tile_pool(name="qkv", bufs=1) as qkvp:
            ident_sb = constp.tile([128, 128], F32R)
            nc.sync.dma_start(ident_sb[:], ident[:, :])
            sgn_sb = constp.tile([128, 1], F32)
            nc.sync.dma_start(sgn_sb[:], sgn[:, :])

            # persistent roped projections + V in natural layout
            qT0 = qkvp.tile([128, T], F32R, tag="qT0")
            qT1 = qkvp.tile([128, T], F32R, tag="qT1")
            kT = qkvp.tile([128, T], F32R, tag="kT")
            vT = qkvp.tile([128, T], F32R, tag="vT")
            chunks = [qT0, qT1, kT]

            # ------ phase 1: projections + inline RoPE + V transpose ------
            with tc.tile_pool(name="w", bufs=1) as wp, \
                 tc.tile_pool(name="cs", bufs=1) as csp, \
                 tc.tile_pool(name="xt", bufs=7) as xtp, \
                 tc.tile_pool(name="rtmp", bufs=1) as rp, \
                 tc.tile_pool(name="pproj", bufs=1, space="PSUM") as pp:
                wq_sb = wp.tile([128, NKD * QW], F32R)
                wk_sb = wp.tile([128, NKD * HD], F32R)
                wv_sb = wp.tile([128, NKD * HD], F32R)
                nc.sync.dma_start(wq_sb.rearrange("p (n m) -> p n m", n=NKD),
                                  wq_c.rearrange("(n p) m -> p n m", p=128))
                nc.sync.dma_start(wk_sb.rearrange("p (n m) -> p n m", n=NKD),
                                  wk_c.rearrange("(n p) m -> p n m", p=128))
                nc.sync.dma_start(wv_sb.rearrange("p (n m) -> p n m", n=NKD),
                                  wv_c.rearrange("(n p) m -> p n m", p=128))
                def w_slice(c, kk):
                    if c < 2:
                        return wq_sb[:, kk * QW + c * 128: kk * QW + (c + 1) * 128]
                    if c == 2:
                        return wk_sb[:, kk * HD:(kk + 1) * HD]
                    return wv_sb[:, kk * HD:(kk + 1) * HD]

                xT3 = xT.rearrange("(n p) m -> p n m", p=128)  # [128,NKD,T]
                npair = T // 1024
                cos_sb = sin_sb = None
                for p in range(npair):
                    # stream this pair's xT as quarter tiles (kk-major use)
                    xts = [[], []]          # [grp][quarter]
                    for q in range(NQT):
                        for j, g in enumerate((2 * p, 2 * p + 1)):
                            xt_q = xtp.tile([128, NKQ * 512], F32R, tag="xt")
                            nc.sync.dma_start(
                                xt_q.rearrange("p (n m) -> p n m", n=NKQ),
                                xT3[:, q * NKQ:(q + 1) * NKQ,
                                    g * 512:(g + 1) * 512])
                            xts[j].append(xt_q)
                    if cos_sb is None:
                        cos_sb = csp.tile([128, SB], F32)
                        nc.sync.dma_start(cos_sb[:], cosd[:, :])
                        sin_sb = csp.tile([128, SB], F32)
                        nc.sync.dma_start(sin_sb[:], sind[:, :])
                    pss = []
                    for c in range(4):
                        ps_c = pp.tile([128, 1024], F32, tag=f"pp{c}")
                        pss.append(ps_c)
                    for kk in range(NKD):
                        for c in range(4):
                            lhsT = w_slice(c, kk)
                            for j in (0, 1):
                                nc.tensor.matmul(
                                    pss[c][:, j * 512:(j + 1) * 512], lhsT,
                                    xts[j][kk // NKQ][:, (kk % NKQ) * 512:
                                                      (kk % NKQ + 1) * 512],
                                    start=(kk == 0), stop=(kk == NKD - 1))
                    # drain q0/q1/k with RoPE staged below; v via transpose
                    cp0 = 1024 * p
                    for c in range(3):
                        nc.vector.tensor_copy(
                            chunks[c][:, cp0:cp0 + 1024], pss[c][:])
                    nc.vector.tensor_copy(vT[:, cp0:cp0 + 1024], pss[3][:])
                    # RoPE on the pair's columns, per batch segment
                    for s0 in range(cp0, cp0 + 1024, SEG):
                        pos0 = s0 % SB
                        for X in chunks:
                            tcs = rp.tile([128, SEG], F32, tag="tc")
                            nc.vector.tensor_tensor(
                                tcs[:], X[:, s0:s0 + SEG],
                                cos_sb[:, pos0:pos0 + SEG],
                                op=mybir.AluOpType.mult)
                            tsn = rp.tile([128, SEG], F32, tag="ts")
                            nc.vector.tensor_tensor(
                                tsn[:], X[:, s0:s0 + SEG],
                                sin_sb[:, pos0:pos0 + SEG],
                                op=mybir.AluOpType.mult)
                            tsw = rp.tile([128, SEG], F32, tag="tw")
                            nc.sync.dma_start(tsw[0:64, :], tsn[64:128, :])
                            nc.sync.dma_start(tsw[64:128, :], tsn[0:64, :])
                            # X = tcs + sgn * tsw   (sgn = -1 top / +1 bottom)
                            nc.vector.scalar_tensor_tensor(
                                X[:, s0:s0 + SEG], tsw[:], sgn_sb[:, 0:1],
                                tcs[:], op0=mybir.AluOpType.mult,
                                op1=mybir.AluOpType.add)

            # ---------------- phase 3: attention ----------------------
            DQ = DIM // 4
            wo3 = wo_f.rearrange("(n p) m -> p n m", p=128)  # [128,NHD,DIM]
            wop = tc.alloc_tile_pool(name="wop", bufs=2)
            wo_half = []
            with tc.tile_pool(name="att", bufs=2) as ap, \
                 tc.tile_pool(name="attc", bufs=1) as apc, \
                 tc.tile_pool(name="pstr", bufs=2) as pstr, \
                 tc.tile_pool(name="psS", bufs=2, space="PSUM") as psS, \
                 tc.tile_pool(name="psO", bufs=1, space="PSUM") as psO:
                wo_sb0 = wop.tile([128, NHD * DQ], F32R, tag="wo")
                nc.sync.dma_start(
                    wo_sb0.rearrange("p (n m) -> p n m", n=NHD),
                    wo3[:, :, 0:DQ])
                wo_half.append(wo_sb0)
                tri_sb = apc.tile([128, 512], F32)
                nc.sync.dma_start(tri_sb[:], tri[:, :])
                ones_sb = apc.tile([128, 128], F32R)
                nc.sync.dma_start(ones_sb[:], ones[:, :])
                Vt = qkvp.tile([128, T], F32R, tag="Vt")
                for ttg in range(NTT):
                    psv = psS.tile([128, 128], F32R, tag="S")
                    nc.tensor.transpose(psv[:],
                                        vT[:, ttg * 128:(ttg + 1) * 128],
                                        ident_sb[:])
                    nc.vector.tensor_copy(Vt[:, ttg * 128:(ttg + 1) * 128],
                                          psv[:])
                for hl in range(HPC):
                    qTh = qT0 if hl == 0 else qT1
                    for b in range(B):
                        qb = b * SB     # q-col base for this batch
                        pO = psO.tile([128, SB], F32, tag="O")
                        acc = ap.tile([128, SB], F32R, tag="acc")
                        for t in range(KT):
                            col0 = 128 * t
                            d = t % 4
                            g0 = t // 4
                            lhsK = kT[:, qb + col0: qb + col0 + 128]
                            bnd = min(1024, SB)
                            tiles = []   # (stile, base, lo, hi)
                            if col0 < bnd:
                                s1 = psS.tile([128, 1024], F32, tag="S")
                                tiles.append((s1, 512 * g0, col0, bnd))
                            if SB > 1024:
                                s2 = psS.tile([128, 1024], F32, tag="S")
                                b2 = max(1024, 512 * g0)
                                tiles.append((s2, b2, max(col0, 1024), SB))
                            for (stile, base, lo, hi) in tiles:
                                for g in range(g0, NG):
                                    glo = max(512 * g, col0)
                                    ghi = 512 * (g + 1)
                                    if ghi <= lo or glo >= hi:
                                        continue
                                    nc.tensor.matmul(
                                        stile[:, glo - base: ghi - base],
                                        lhsK,
                                        qTh[:, qb + glo: qb + ghi],
                                        start=True, stop=True)
                            # exp -> P strip (f32r)
                            P = pstr.tile([128, SB], F32R, tag="P")
                            for (stile, base, lo, hi) in tiles:
                                nc.scalar.activation(
                                    P[:, lo - col0: hi - col0],
                                    stile[:, lo - base: hi - base],
                                    mybir.ActivationFunctionType.Exp,
                                    scale=SCALE)
                            # causal mask on the diagonal block
                            dw = 512 - 128 * d
                            nc.vector.tensor_tensor(
                                P[:, 0:dw], P[:, 0:dw], tri_sb[:, 0:dw],
                                op=mybir.AluOpType.mult)
                            # accumulate exp sums
                            if t == 0:
                                nc.vector.tensor_copy(acc[:], P[:])
                            else:
                                nc.vector.tensor_tensor(
                                    acc[:, col0:SB], acc[:, col0:SB],
                                    P[:, 0:SB - col0],
                                    op=mybir.AluOpType.add)
                            # P @ V accumulation into O^T
                            lhsV = Vt[:, (b * KT + t) * 128:
                                      (b * KT + t + 1) * 128]
                            for g in range(g0, NG):
                                glo = max(512 * g, col0)
                                ghi = 512 * (g + 1)
                                nc.tensor.matmul(
                                    pO[:, glo:ghi], lhsV,
                                    P[:, glo - col0: ghi - col0],
                                    start=(t == 0),
                                    stop=(t == 4 * g + 3))
                        # epilogue: broadcast sums, fast reciprocal, scale
                        Ofin = ap.tile([128, SB], F32R, tag="Of")
                        for g in range(NG):
                            psr = psS.tile([128, 512], F32, tag="S")
                            nc.tensor.matmul(psr[:], ones_sb[:],
                                             acc[:, 512 * g:512 * (g + 1)],
                                             start=True, stop=True)
                            rb = ap.tile([128, 512], F32, tag="rb")
                            scr = ap.tile([128, 512], F32, tag="scr")
                            nc.vector.reciprocal_approx_accurate(
                                rb[:], psr[:], scr[:])
                            nc.vector.tensor_tensor(
                                Ofin[:, 512 * g:512 * (g + 1)],
                                pO[:, 512 * g:512 * (g + 1)], rb[:],
                                op=mybir.AluOpType.mult)
                        # ship this (b, head) to its a2a dest slots
                        nd = SB // TPC
                        d0 = (b * SB) // TPC
                        for s in range(nd):
                            nc.sync.dma_start(
                                a2a_in[hl][d0 + s, :, :],
                                Ofin[:, s * TPC:(s + 1) * TPC])
                    # per-head collective, overlaps the next head's attention
                    nc.gpsimd.collective_compute(
                        "AllToAll", mybir.AluOpType.bypass,
                        replica_groups=[list(range(N_CORES))],
                        ins=[a2a_in[hl].opt()], outs=[a2a_out[hl].opt()])

        # ---------------- phase 5: O-projection ----------------------
            kks0 = list(range(0, NHD, HPC))      # head-0 hd tiles
            kks1 = list(range(1, NHD, HPC)) if HPC > 1 else []
            with tc.tile_pool(name="oproj", bufs=1) as op, \
                 tc.tile_pool(name="ostg", bufs=2) as ostg, \
                 tc.tile_pool(name="psop", bufs=8, space="PSUM") as pso:
                recv = {}
                for kk in kks0 + kks1:
                    rv = op.tile([128, TPC], F32R, tag=f"rv{kk}")
                    nc.sync.dma_start(rv[:], a2a_out[kk % HPC][kk // HPC, :, :])
                    recv[kk] = rv
                NQO = DIM // DQ
                NTO = TPC // 128
                for wave in range(max(1, NQO // 2)):
                    qs = [q for q in (2 * wave, 2 * wave + 1) if q < NQO]
                    wos = {}
                    for q in qs:
                        if q == 0:
                            wos[q] = wo_half[0]
                        else:
                            wo_sb = wop.tile([128, NHD * DQ], F32R, tag="wo")
                            nc.sync.dma_start(
                                wo_sb.rearrange("p (n m) -> p n m", n=NHD),
                                wo3[:, :, q * DQ:(q + 1) * DQ])
                            wos[q] = wo_sb
                    po_map = {}
                    for q in qs:
                        for tt in range(NTO):
                            po = pso.tile([128, DQ], F32, tag="po")
                            po_map[(q, tt)] = po
                            for ki, kk in enumerate(kks0):
                                nc.tensor.matmul(
                                    po[:], recv[kk][:, tt * 128:(tt + 1) * 128],
                                    wos[q][:, kk * DQ:(kk + 1) * DQ],
                                    start=(ki == 0),
                                    stop=(not kks1 and ki == len(kks0) - 1),
                                    skip_group_check=True)
                    for q in qs:
                        for tt in range(NTO):
                            po = po_map[(q, tt)]
                            for ki, kk in enumerate(kks1):
                                nc.tensor.matmul(
                                    po[:], recv[kk][:, tt * 128:(tt + 1) * 128],
                                    wos[q][:, kk * DQ:(kk + 1) * DQ],
                                    start=False, stop=(ki == len(kks1) - 1),
                                    skip_group_check=True)
                            stg = ostg.tile([128, DQ], F32, tag="stg")
                            nc.vector.tensor_copy(stg[:], po[:])
                            nc.sync.dma_start(
                                out_c[tt * 128:(tt + 1) * 128,
                                      q * DQ:(q + 1) * DQ], stg[:])
            wop.release()

    if not nc.is_finalized():
        nc.finalize()
    return nc


_NC_CACHE = {}


def _get_nc():
    if "nc" not in _NC_CACHE:
        _NC_CACHE["nc"] = _build()
    return _NC_CACHE["nc"]


def _prep_inputs(x, cos, sin, wq, wk, wv, wo):
    x = np.asarray(x, np.float32)
    cos = np.asarray(cos, np.float32)
    sin = np.asarray(sin, np.float32)
    wq = np.asarray(wq, np.float32)
    wk = np.asarray(wk, np.float32)
    wv = np.asarray(wv, np.float32)
    wo = np.asarray(wo, np.float32)

    xT = np.ascontiguousarray(x.reshape(T, DIM).T)
    perm = np.r_[np.arange(0, HD, 2), np.arange(1, HD, 2)]
    wq_sh = wq.reshape(DIM, H, HD)[:, :, perm]
    wk_sh = wk.reshape(DIM, HKV, HD)[:, :, perm]
    wv_r = wv.reshape(DIM, HKV, HD)
    cosT = np.ascontiguousarray(cos.T)          # [64, SB]
    cosd_a = np.vstack([cosT, cosT])            # [128, SB]
    sinT = np.ascontiguousarray(sin.T)
    sind_a = np.vstack([sinT, sinT])
    sgn_a = np.vstack([np.full((64, 1), -1.0, np.float32),
                       np.full((64, 1), 1.0, np.float32)])
    tri_a = (np.arange(512)[None, :] >= np.arange(128)[:, None]
             ).astype(np.float32)
    ones_a = np.ones((128, 128), np.float32)
    ident_a = np.eye(128, dtype=np.float32)

    in_maps = []
    for c in range(N_CORES):
        h0 = HPC * c
        g = h0 // (H // HKV)
        in_maps.append({
            "xT": xT,
            "wq_c": np.ascontiguousarray(
                wq_sh[:, h0:h0 + HPC].reshape(DIM, QW)),
            "wk_c": np.ascontiguousarray(wk_sh[:, g]),
            "wv_c": np.ascontiguousarray(wv_r[:, g]),
            "wo_f": wo,
            "cosd": cosd_a, "sind": sind_a, "sgn": sgn_a, "tri": tri_a,
            "ones": ones_a, "ident": ident_a,
        })
    return in_maps


def _run(inputs, trace=False):
    in_maps = _prep_inputs(**inputs)
    nc = _get_nc()
    res = run_bass_kernel_spmd(
        nc, in_maps, core_ids=list(range(N_CORES)), trace=trace,
        trace_cores=list(range(N_CORES)) if trace else None)
    out = np.concatenate([res.results[c]["out_c"] for c in range(N_CORES)],
                         axis=0)
    return out.reshape(B, SB, DIM), res


def kernel(**inputs):
    out, _ = _run(inputs, trace=os.environ.get("KERNEL_TRACE", "0") == "1")
    return out



# revision 2
# speedup vs baseline: 1.2196x; 1.2196x over previous
"""GQA attention + RoPE + O-proj, tensor-parallel over 8 NeuronCores.

v2: fp16 operands for all matmuls (fp32r runs 2-pass "HIGH" mode on HW;
fp16 streams 1 cycle/row and halves LDWEIGHTS + DMA + collective bytes).

Strategy (head-parallel TP + all-to-all reshard before O-proj):
  - host: transpose x -> xT [DIM, T] fp16; shuffle per-head wq/wk columns
    to [even hd | odd hd] so RoPE works in the transposed layout; fold the
    rotation sign into sinS = [+sin; -sin].
  - core c: projects q for heads {2c, 2c+1} and k,v for kv-head c//2 over
    all tokens (weight-stationary fp16 matmuls, xT streamed in quarter
    tiles), drains PSUM to fp16, applies RoPE on fp16 tiles (DVE 2x for q,
    gpsimd for k), transposes V inline, then runs causal attention in S^T
    [k, q] layout with no-max softmax (scores ~N(0,1)); denominators via
    fp16 strip accumulation + ones-matmul broadcast + Newton reciprocal.
  - Two AllToAlls (one per local head, fp16, overlapped with attention)
    reshard attention outputs head-major -> token-sharded; each core then
    computes its 512 output rows against the full wo (double-buffered).
"""

import os
import numpy as np

import concourse.bass as bass
import concourse.bacc as bacc
import concourse.tile as tile
from concourse import mybir
from concourse.bass_utils import run_bass_kernel_spmd

F32 = mybir.dt.float32
F16 = mybir.dt.float16

N_CORES = 8

# Full-problem config (hardcoded per spec).
B, SB, DIM = 2, 2048, 2048         # batches, seq per batch, model dim
H, HKV, HD = 16, 4, 128            # q heads, kv heads, head dim
SCALE = 1.0 / float(np.sqrt(HD))

T = B * SB                          # 4096 flat tokens (batch-major)
TPC = T // N_CORES                  # 512 tokens per core (output shard)
HPC = H // N_CORES                  # 2 q heads per core
QW = HPC * HD                       # 256 q cols per core
NKD = DIM // 128                    # 16 contraction tiles for projections
NG = SB // 512                      # 4 q-groups of 512 per batch
KT = SB // 128                      # 16 k-tiles per batch
NTT = T // 128                      # 32 token tiles total
NHD = (H * HD) // 128               # 16 hd row-tiles of wo


def _build():
    nc = bacc.Bacc("TRN2", target_bir_lowering=False, debug=False,
                   num_devices=N_CORES)

    xT = nc.dram_tensor("xT", [DIM, T], F16, kind="ExternalInput").ap()
    wq_c = nc.dram_tensor("wq_c", [DIM, QW], F16, kind="ExternalInput").ap()
    wk_c = nc.dram_tensor("wk_c", [DIM, HD], F16, kind="ExternalInput").ap()
    wv_c = nc.dram_tensor("wv_c", [DIM, HD], F16, kind="ExternalInput").ap()
    wo_f = nc.dram_tensor("wo_f", [H * HD, DIM], F16, kind="ExternalInput").ap()
    cosd = nc.dram_tensor("cosd", [128, SB], F16, kind="ExternalInput").ap()
    sind = nc.dram_tensor("sind", [128, SB], F16, kind="ExternalInput").ap()
    tri = nc.dram_tensor("tri", [128, 128], F16, kind="ExternalInput").ap()
    ones = nc.dram_tensor("ones", [128, 128], F16, kind="ExternalInput").ap()
    ident = nc.dram_tensor("ident", [128, 128], F16, kind="ExternalInput").ap()
    out_c = nc.dram_tensor("out_c", [TPC, DIM], F32, kind="ExternalOutput").ap()

    a2a_in = []
    a2a_out = []
    for hl in range(HPC):
        a2a_in.append(nc.dram_tensor(f"a2a_in{hl}",
                                     [N_CORES, HD, TPC], F16).ap())
        a2a_out.append(nc.dram_tensor(f"a2a_out{hl}",
                                      [N_CORES, HD, TPC], F16).ap())

    NKQ = max(1, NKD // 4)         # dim-tiles per xt quarter
    NQT = NKD // NKQ               # quarters per token group

    with tile.TileContext(nc) as tc:
        with tc.tile_pool(name="const", bufs=1) as constp, \
             tc.tile_pool(name="qkv", bufs=1) as qkvp:
            ident_sb = constp.tile([128, 128], F16)
            nc.sync.dma_start(ident_sb[:], ident[:, :])

            # persistent roped projections + V in natural layout (fp16)
            qT0 = qkvp.tile([128, T], F16, tag="qT0")
            qT1 = qkvp.tile([128, T], F16, tag="qT1")
            kT = qkvp.tile([128, T], F16, tag="kT")
            vT = qkvp.tile([128, T], F16, tag="vT")
            chunks = [qT0, qT1, kT]

            # ------ phase 1: projections + RoPE + V transpose ------
            with tc.tile_pool(name="w", bufs=1) as wp, \
                 tc.tile_pool(name="cs", bufs=1) as csp, \
                 tc.tile_pool(name="xt", bufs=7) as xtp, \
                 tc.tile_pool(name="rtmp", bufs=2) as rp, \
                 tc.tile_pool(name="pproj", bufs=1, space="PSUM") as pp:
                wq_sb = wp.tile([128, NKD * QW], F16)
                wk_sb = wp.tile([128, NKD * HD], F16)
                wv_sb = wp.tile([128, NKD * HD], F16)
                nc.sync.dma_start(wq_sb.rearrange("p (n m) -> p n m", n=NKD),
                                  wq_c.rearrange("(n p) m -> p n m", p=128))
                nc.sync.dma_start(wk_sb.rearrange("p (n m) -> p n m", n=NKD),
                                  wk_c.rearrange("(n p) m -> p n m", p=128))
                nc.sync.dma_start(wv_sb.rearrange("p (n m) -> p n m", n=NKD),
                                  wv_c.rearrange("(n p) m -> p n m", p=128))
                def w_slice(c, kk):
                    if c < 2:
                        return wq_sb[:, kk * QW + c * 128: kk * QW + (c + 1) * 128]
                    if c == 2:
                        return wk_sb[:, kk * HD:(kk + 1) * HD]
                    return wv_sb[:, kk * HD:(kk + 1) * HD]

                xT3 = xT.rearrange("(n p) m -> p n m", p=128)  # [128,NKD,T]
                npair = T // 1024
                cos_sb = sin_sb = None
                for p in range(npair):
                    # stream this pair's xT as quarter tiles (kk-major use)
                    xts = [[], []]          # [grp][quarter]
                    for q in range(NQT):
                        for j, g in enumerate((2 * p, 2 * p + 1)):
                            xt_q = xtp.tile([128, NKQ * 512], F16, tag="xt")
                            nc.sync.dma_start(
                                xt_q.rearrange("p (n m) -> p n m", n=NKQ),
                                xT3[:, q * NKQ:(q + 1) * NKQ,
                                    g * 512:(g + 1) * 512])
                            xts[j].append(xt_q)
                    if cos_sb is None:
                        cos_sb = csp.tile([128, SB], F16)
                        nc.sync.dma_start(cos_sb[:], cosd[:, :])
                        sin_sb = csp.tile([128, SB], F16)
                        nc.sync.dma_start(sin_sb[:], sind[:, :])
                    pss = []
                    for c in range(4):
                        ps_c = pp.tile([128, 1024], F32, tag=f"pp{c}")
                        pss.append(ps_c)
                    for kk in range(NKD):
                        for c in range(4):
                            lhsT = w_slice(c, kk)
                            for j in (0, 1):
                                nc.tensor.matmul(
                                    pss[c][:, j * 512:(j + 1) * 512], lhsT,
                                    xts[j][kk // NKQ][:, (kk % NKQ) * 512:
                                                      (kk % NKQ + 1) * 512],
                                    start=(kk == 0), stop=(kk == NKD - 1))
                    # drain all chunks to fp16 (q0/q1 on DVE, k/v on Act)
                    cp0 = 1024 * p
                    nc.vector.tensor_copy(qT0[:, cp0:cp0 + 1024], pss[0][:])
                    nc.vector.tensor_copy(qT1[:, cp0:cp0 + 1024], pss[1][:])
                    nc.scalar.copy(kT[:, cp0:cp0 + 1024], pss[2][:])
                    nc.scalar.copy(vT[:, cp0:cp0 + 1024], pss[3][:])
                    # RoPE on fp16 tiles: X = X*cos + swap(X*sinS)
                    pos0 = cp0 % SB
                    for ci, X in enumerate(chunks):
                        eng = nc.gpsimd if ci == 2 else nc.vector
                        tcs = rp.tile([128, 1024], F16, tag="tc")
                        eng.tensor_tensor(
                            tcs[:], X[:, cp0:cp0 + 1024],
                            cos_sb[:, pos0:pos0 + 1024],
                            op=mybir.AluOpType.mult)
                        tsn = rp.tile([128, 1024], F16, tag="ts")
                        eng.tensor_tensor(
                            tsn[:], X[:, cp0:cp0 + 1024],
                            sin_sb[:, pos0:pos0 + 1024],
                            op=mybir.AluOpType.mult)
                        tsw = rp.tile([128, 1024], F16, tag="tw")
                        nc.sync.dma_start(tsw[0:64, :], tsn[64:128, :])
                        nc.sync.dma_start(tsw[64:128, :], tsn[0:64, :])
                        eng.tensor_tensor(
                            X[:, cp0:cp0 + 1024], tcs[:], tsw[:],
                            op=mybir.AluOpType.add)

            # ---------------- phase 3: attention ----------------------
            DQ = DIM // 4
            wo3 = wo_f.rearrange("(n p) m -> p n m", p=128)  # [128,NHD,DIM]
            wop = tc.alloc_tile_pool(name="wop", bufs=2)
            wo_half = []
            with tc.tile_pool(name="att", bufs=2) as ap, \
                 tc.tile_pool(name="attc", bufs=1) as apc, \
                 tc.tile_pool(name="pstr", bufs=2) as pstr, \
                 tc.tile_pool(name="psS", bufs=2, space="PSUM") as psS, \
                 tc.tile_pool(name="psO", bufs=1, space="PSUM") as psO:
                wo_sb0 = wop.tile([128, NHD * DQ], F16, tag="wo")
                nc.sync.dma_start(
                    wo_sb0.rearrange("p (n m) -> p n m", n=NHD),
                    wo3[:, :, 0:DQ])
                wo_half.append(wo_sb0)
                tri_sb = apc.tile([128, 128], F16)
                nc.sync.dma_start(tri_sb[:], tri[:, :])
                ones_sb = apc.tile([128, 128], F16)
                nc.sync.dma_start(ones_sb[:], ones[:, :])
                Vt = qkvp.tile([128, T], F16, tag="Vt")
                for ttg in range(NTT):
                    psv = psS.tile([128, 128], F16, tag="S")
                    nc.tensor.transpose(psv[:],
                                        vT[:, ttg * 128:(ttg + 1) * 128],
                                        ident_sb[:])
                    nc.vector.tensor_copy(Vt[:, ttg * 128:(ttg + 1) * 128],
                                          psv[:])
                for hl in range(HPC):
                    qTh = qT0 if hl == 0 else qT1
                    for b in range(B):
                        qb = b * SB     # q-col base for this batch
                        pO = psO.tile([128, SB], F32, tag="O")
                        acc = ap.tile([128, SB], F16, tag="acc")
                        for t in range(KT):
                            col0 = 128 * t
                            d = t % 4
                            g0 = t // 4
                            lhsK = kT[:, qb + col0: qb + col0 + 128]
                            bnd = min(1024, SB)
                            tiles = []   # (stile, base, lo, hi)
                            if col0 < bnd:
                                s1 = psS.tile([128, 1024], F32, tag="S")
                                tiles.append((s1, 512 * g0, col0, bnd))
                            if SB > 1024:
                                s2 = psS.tile([128, 1024], F32, tag="S")
                                b2 = max(1024, 512 * g0)
                                tiles.append((s2, b2, max(col0, 1024), SB))
                            for (stile, base, lo, hi) in tiles:
                                for g in range(g0, NG):
                                    glo = max(512 * g, col0)
                                    ghi = 512 * (g + 1)
                                    if ghi <= lo or glo >= hi:
                                        continue
                                    nc.tensor.matmul(
                                        stile[:, glo - base: ghi - base],
                                        lhsK,
                                        qTh[:, qb + glo: qb + ghi],
                                        start=True, stop=True)
                            # exp -> P strip (fp16)
                            P = pstr.tile([128, SB], F16, tag="P")
                            for (stile, base, lo, hi) in tiles:
                                nc.scalar.activation(
                                    P[:, lo - col0: hi - col0],
                                    stile[:, lo - base: hi - base],
                                    mybir.ActivationFunctionType.Exp,
                                    scale=SCALE)
                            # causal mask on the 128-wide diagonal block
                            nc.vector.tensor_tensor(
                                P[:, 0:128], P[:, 0:128], tri_sb[:],
                                op=mybir.AluOpType.mult)
                            # accumulate exp sums (fp16, DVE 2x)
                            if t == 0:
                                nc.vector.tensor_copy(acc[:], P[:])
                            else:
                                nc.vector.tensor_tensor(
                                    acc[:, col0:SB], acc[:, col0:SB],
                                    P[:, 0:SB - col0],
                                    op=mybir.AluOpType.add)
                            # P @ V accumulation into O^T
                            lhsV = Vt[:, (b * KT + t) * 128:
                                      (b * KT + t + 1) * 128]
                            for g in range(g0, NG):
                                glo = max(512 * g, col0)
                                ghi = 512 * (g + 1)
                                nc.tensor.matmul(
                                    pO[:, glo:ghi], lhsV,
                                    P[:, glo - col0: ghi - col0],
                                    start=(t == 0),
                                    stop=(t == 4 * g + 3))
                        # epilogue: broadcast sums, fast reciprocal, scale
                        Ofin = ap.tile([128, SB], F16, tag="Of")
                        for g in range(NG):
                            psr = psS.tile([128, 512], F32, tag="S")
                            nc.tensor.matmul(psr[:], ones_sb[:],
                                             acc[:, 512 * g:512 * (g + 1)],
                                             start=True, stop=True)
                            rb = ap.tile([128, 512], F32, tag="rb")
                            scr = ap.tile([128, 512], F32, tag="scr")
                            nc.vector.reciprocal_approx_accurate(
                                rb[:], psr[:], scr[:])
                            nc.vector.tensor_tensor(
                                Ofin[:, 512 * g:512 * (g + 1)],
                                pO[:, 512 * g:512 * (g + 1)], rb[:],
                                op=mybir.AluOpType.mult)
                        # ship this (b, head) to its a2a dest slots
                        nd = SB // TPC
                        d0 = (b * SB) // TPC
                        for s in range(nd):
                            nc.sync.dma_start(
                                a2a_in[hl][d0 + s, :, :],
                                Ofin[:, s * TPC:(s + 1) * TPC])
                    # per-head collective, overlaps the next head's attention
                    nc.gpsimd.collective_compute(
                        "AllToAll", mybir.AluOpType.bypass,
                        replica_groups=[list(range(N_CORES))],
                        ins=[a2a_in[hl].opt()], outs=[a2a_out[hl].opt()])

        # ---------------- phase 5: O-projection ----------------------
            kks0 = list(range(0, NHD, HPC))      # head-0 hd tiles
            kks1 = list(range(1, NHD, HPC)) if HPC > 1 else []
            with tc.tile_pool(name="oproj", bufs=1) as op, \
                 tc.tile_pool(name="ostg", bufs=2) as ostg, \
                 tc.tile_pool(name="psop", bufs=8, space="PSUM") as pso:
                recv = {}
                for kk in kks0 + kks1:
                    rv = op.tile([128, TPC], F16, tag=f"rv{kk}")
                    nc.sync.dma_start(rv[:], a2a_out[kk % HPC][kk // HPC, :, :])
                    recv[kk] = rv
                NQO = DIM // DQ
                NTO = TPC // 128
                for wave in range(max(1, NQO // 2)):
                    qs = [q for q in (2 * wave, 2 * wave + 1) if q < NQO]
                    wos = {}
                    for q in qs:
                        if q == 0:
                            wos[q] = wo_half[0]
                        else:
                            wo_sb = wop.tile([128, NHD * DQ], F16, tag="wo")
                            nc.sync.dma_start(
                                wo_sb.rearrange("p (n m) -> p n m", n=NHD),
                                wo3[:, :, q * DQ:(q + 1) * DQ])
                            wos[q] = wo_sb
                    po_map = {}
                    for q in qs:
                        for tt in range(NTO):
                            po = pso.tile([128, DQ], F32, tag="po")
                            po_map[(q, tt)] = po
                            for ki, kk in enumerate(kks0):
                                nc.tensor.matmul(
                                    po[:], recv[kk][:, tt * 128:(tt + 1) * 128],
                                    wos[q][:, kk * DQ:(kk + 1) * DQ],
                                    start=(ki == 0),
                                    stop=(not kks1 and ki == len(kks0) - 1),
                                    skip_group_check=True)
                    for q in qs:
                        for tt in range(NTO):
                            po = po_map[(q, tt)]
                            for ki, kk in enumerate(kks1):
                                nc.tensor.matmul(
                                    po[:], recv[kk][:, tt * 128:(tt + 1) * 128],
                                    wos[q][:, kk * DQ:(kk + 1) * DQ],
                                    start=False, stop=(ki == len(kks1) - 1),
                                    skip_group_check=True)
                            stg = ostg.tile([128, DQ], F32, tag="stg")
                            nc.vector.tensor_copy(stg[:], po[:])
                            nc.sync.dma_start(
                                out_c[tt * 128:(tt + 1) * 128,
                                      q * DQ:(q + 1) * DQ], stg[:])
            wop.release()

    if not nc.is_finalized():
        nc.finalize()
    return nc


_NC_CACHE = {}


def _get_nc():
    if "nc" not in _NC_CACHE:
        _NC_CACHE["nc"] = _build()
    return _NC_CACHE["nc"]


def _prep_inputs(x, cos, sin, wq, wk, wv, wo):
    x = np.asarray(x, np.float32)
    cos = np.asarray(cos, np.float32)
    sin = np.asarray(sin, np.float32)
    wq = np.asarray(wq, np.float32)
    wk = np.asarray(wk, np.float32)
    wv = np.asarray(wv, np.float32)
    wo = np.asarray(wo, np.float32)

    xT = np.ascontiguousarray(x.reshape(T, DIM).T).astype(np.float16)
    perm = np.r_[np.arange(0, HD, 2), np.arange(1, HD, 2)]
    wq_sh = wq.reshape(DIM, H, HD)[:, :, perm].astype(np.float16)
    wk_sh = wk.reshape(DIM, HKV, HD)[:, :, perm].astype(np.float16)
    wv_r = wv.reshape(DIM, HKV, HD).astype(np.float16)
    wo_h = wo.astype(np.float16)
    cosT = np.ascontiguousarray(cos.T)          # [64, SB]
    cosd_a = np.vstack([cosT, cosT]).astype(np.float16)
    sinT = np.ascontiguousarray(sin.T)
    # sign folded: out = X*cos + swap(X*sinS), sinS = [+sin; -sin]
    sind_a = np.vstack([sinT, -sinT]).astype(np.float16)
    tri_a = (np.arange(128)[None, :] >= np.arange(128)[:, None]
             ).astype(np.float16)
    ones_a = np.ones((128, 128), np.float16)
    ident_a = np.eye(128, dtype=np.float16)

    in_maps = []
    for c in range(N_CORES):
        h0 = HPC * c
        g = h0 // (H // HKV)
        in_maps.append({
            "xT": xT,
            "wq_c": np.ascontiguousarray(
                wq_sh[:, h0:h0 + HPC].reshape(DIM, QW)),
            "wk_c": np.ascontiguousarray(wk_sh[:, g]),
            "wv_c": np.ascontiguousarray(wv_r[:, g]),
            "wo_f": wo_h,
            "cosd": cosd_a, "sind": sind_a, "tri": tri_a,
            "ones": ones_a, "ident": ident_a,
        })
    return in_maps


def _run(inputs, trace=False):
    in_maps = _prep_inputs(**inputs)
    nc = _get_nc()
    res = run_bass_kernel_spmd(
        nc, in_maps, core_ids=list(range(N_CORES)), trace=trace,
        trace_cores=list(range(N_CORES)) if trace else None)
    out = np.concatenate([res.results[c]["out_c"] for c in range(N_CORES)],
                         axis=0)
    return out.astype(np.float32).reshape(B, SB, DIM), res


def kernel(**inputs):
    out, _ = _run(inputs, trace=os.environ.get("KERNEL_TRACE", "0") == "1")
    return out
